# revision 1
# baseline (speedup 1.0000x reference)
"""Expert-parallel MoE layer for 8 Trainium2 NeuronCores.

Strategy: each of the 8 experts is assigned to one core. The host computes
the routing (which tokens go to which expert and with what combined weight),
gathers + transposes each expert's tokens into bf16 device tensors that are
pre-packed in the exact SBUF consumption layout (partition-major), and each
core runs a fused  gelu(x @ W1 + b1) @ W2  kernel for its expert. The host
applies the per-token combine weight and the (zero-ish) b2 term while
scatter-adding the per-expert outputs back into the full [B, S, D] output,
so neither cw nor b2 ever travels to the device.

Why pre-pack: a [D, C] activation layout gives the DMA engines 1KB runs and
512-row descriptors per tile; packed [128, DC, ssz] layouts give 128-row
descriptors with multi-KB contiguous runs per partition, which is what the
head of the kernel (PE waiting on the first tiles) is bound by. Tensor count
is kept low (5) because each bound DRAM tensor adds launch preamble time.

All matmul operands and the output travel as bf16 (norm rel err ~4e-3,
budget 2e-2): halves DMA bytes and SBUF pressure vs f32r at the same
1 column/cycle PE rate.

Schedule: token columns are processed in nsub equal sub-blocks (<=512 cols,
one PSUM bank) in a phased order -- L1 over the first three subs, then
their L2 sweep, then the remaining subs pipelined -- which keeps early chip
power to one engine class (the 50%-duty power-brake windows fire on
concurrency spikes) and gives w2 a wide landing window. The last sub's L2
runs in half-column passes so the kernel tail drains a short block. Inputs
feed through both HWDGE rings (sync: w1 in per-fc stages / x^T head / w2;
scalar: b1 + late x^T), outputs ride the scalar ring. Zero-operand PE-warmup
matmuls ramp the clock during the DMA wait while keeping pre-stream power
low (hot warmup on all 8 cores provokes a chip-wide downclock).
"""

import sys

if "/opt/trn_rl_repo" not in sys.path:
    sys.path.insert(0, "/opt/trn_rl_repo")

import ml_dtypes
import numpy as np

import concourse.bass as bass
import concourse.tile as tile
from concourse import bacc, mybir
from concourse.bass_utils import run_bass_kernel_spmd

B, S, D, F, E, TOPK = 4, 2048, 512, 1024, 8, 2
T = B * S
F32 = mybir.dt.float32
BF16 = mybir.dt.bfloat16
NPBF16 = ml_dtypes.bfloat16

DC = D // 128  # 4 contraction chunks for x @ W1
FC = F // 128  # 8 contraction chunks for h @ W2

# PE clock-ramp warmup: number of 128-col dummy matmuls issued before the
# first data-dependent matmul (tuned against the trace so the chain ends
# right as the first x^T/W1 slices land; slightly long is safer than idle).
N_WARMUP = 36

# Set by test harness to capture a profile; harness-invisible otherwise.
TRACE = False
LAST_RESULTS = None

_nc_cache = {}


def _grid(C):
    """(nsub, Csub, C_padded): equal sub-blocks of <=512 cols."""
    nsub = -(-C // 512)
    csub = -(-C // (nsub * 16)) * 16
    return nsub, csub, nsub * csub


def _build_nc(C):
    nsub, csub, cpad = _grid(C)
    assert cpad == C, (C, nsub, csub)

    nc = bacc.Bacc("TRN2", num_devices=E)

    xt_d = nc.dram_tensor("xt", [128, nsub, DC, csub], BF16, kind="ExternalInput")
    w1_d = nc.dram_tensor("w1", [128, FC, DC, 128], BF16, kind="ExternalInput")
    w2_d = nc.dram_tensor("w2", [128, DC, FC, 128], BF16, kind="ExternalInput")
    b1_d = nc.dram_tensor("b1", [128, FC], F32, kind="ExternalInput")
    yt_d = nc.dram_tensor("yt", [128, nsub, DC, csub], BF16, kind="ExternalOutput")

    with tile.TileContext(nc) as tc:
        with (
            tc.tile_pool(name="consts", bufs=1) as consts,
            tc.tile_pool(name="xtp", bufs=1) as xtp,
            tc.tile_pool(name="hp", bufs=26) as hp,
            tc.tile_pool(name="ybig", bufs=8) as ybigp,
            tc.tile_pool(name="ps_h", bufs=4, space="PSUM") as ps_h,
            tc.tile_pool(name="ps_y", bufs=4, space="PSUM") as ps_y,
        ):
            wu_w = consts.tile([128, 128], BF16, tag="wu_w")
            nc.vector.memset(wu_w[:, :], 0.0)
            wu_x = consts.tile([128, 128], BF16, tag="wu_x")
            nc.vector.memset(wu_x[:, :], 0.0)
            wu_ps = ps_h.tile([128, 128], F32, tag="psh")
            for k in range(N_WARMUP):
                nc.tensor.matmul(
                    wu_ps[:, :], wu_w[:, :], wu_x[:, :],
                    start=(k == 0), stop=(k == N_WARMUP - 1),
                )

            # ---- sync HWDGE ring: first-gate trio, then w1/w2 staged ----
            w1_sb = consts.tile([128, FC, DC, 128], BF16, tag="w1")
            nc.sync.dma_start(out=w1_sb[:, 0, :, :], in_=w1_d[:, 0, :, :])

            # first sub's x^T as two separate tiles: the dc0/dc1 matmuls of
            # the first fc group gate on the first 256KB alone instead of
            # the whole half-MB sub-block
            xt0a = xtp.tile([128, 2, csub], BF16, tag="xt0a", name="xt_sb0a")
            nc.sync.dma_start(out=xt0a[:, :, :], in_=xt_d[:, 0, 0:2, :])
            xt0b = xtp.tile([128, 2, csub], BF16, tag="xt0b", name="xt_sb0b")
            nc.sync.dma_start(out=xt0b[:, :, :], in_=xt_d[:, 0, 2:4, :])
            xt_sb = {}

            # rest of w1 in per-fc-stage DMAs: the early DMA path runs well
            # below steady-state rate, so each 0.25MB stage lands just ahead
            # of the fc group that consumes it instead of stalling the PE
            # behind one big transfer
            for fc in range(1, FC):
                nc.sync.dma_start(
                    out=w1_sb[:, fc, :, :], in_=w1_d[:, fc, :, :]
                )

            def _load_xt(si, eng):
                t = xtp.tile([128, DC, csub], BF16, tag=f"xt{si}", name=f"xt_sb{si}")
                eng.dma_start(out=t[:, :, :], in_=xt_d[:, si, :, :])
                xt_sb[si] = t

            # x^T for the rest of tile0 (consumed sub-major after s0's fc
            # sweep), then w2 ahead of tile0's L2. Everything the first ~20us
            # consumes stays on the sync ring in strict consumption order --
            # parallel early traffic on the scalar ring measurably starves
            # the w1 feed (shared SDMA/HBM path runs below steady rate).
            for si in range(1, min(3, nsub)):
                _load_xt(si, nc.sync)

            # w2: first d-chunk unblocks L2(s0), then the rest
            w2_sb = consts.tile([128, DC, FC, 128], BF16, tag="w2")
            nc.sync.dma_start(out=w2_sb[:, 0, :, :], in_=w2_d[:, 0, :, :])
            nc.sync.dma_start(out=w2_sb[:, 1:DC, :, :], in_=w2_d[:, 1:DC, :, :])

            # ---- scalar HWDGE ring: b1, late x^T, then output DMAs ----
            b1_sb = consts.tile([128, FC], F32, tag="b1")
            nc.scalar.dma_start(out=b1_sb[:, :], in_=b1_d[:, :])

            for si in range(3, nsub):
                _load_xt(si, nc.scalar)

            h_tiles = {}

            def layer1(si):
                for fc in range(FC):
                    ps = ps_h.tile([128, csub], F32, tag="psh")
                    for dc in range(DC):
                        if si == 0:
                            rhs = (xt0a[:, dc, :] if dc < 2
                                   else xt0b[:, dc - 2, :])
                        else:
                            rhs = xt_sb[si][:, dc, :]
                        nc.tensor.matmul(
                            ps[:, :],
                            w1_sb[:, fc, dc, :],
                            rhs,
                            start=(dc == 0),
                            stop=(dc == DC - 1),
                        )
                    h = hp.tile([128, csub], BF16, tag="h")
                    nc.scalar.activation(
                        h[:, :], ps[:, :],
                        mybir.ActivationFunctionType.Gelu_apprx_tanh,
                        bias=b1_sb[:, fc:fc + 1],
                    )
                    h_tiles[(si, fc)] = h

            def layer2(si, col_split=1):
                # col_split > 1 runs each dc group in column chunks so the
                # final chunk (the kernel tail) is short.
                chunks = []
                step = -(-csub // col_split)
                lo = 0
                while lo < csub:
                    chunks.append((lo, min(step, csub - lo)))
                    lo += step
                for dc in range(DC):
                    for lo, ln in chunks:
                        ps2 = ps_y.tile([128, ln], F32, tag="psy")
                        for fc in range(FC):
                            nc.tensor.matmul(
                                ps2[:, :],
                                w2_sb[:, dc, fc, :],
                                h_tiles[(si, fc)][:, lo:lo + ln],
                                start=(fc == 0),
                                stop=(fc == FC - 1),
                            )
                        # PSUM -> bf16 SBUF on the DVE; combine weights and
                        # b2 are applied on the host during the scatter.
                        yout = ybigp.tile([128, ln], BF16, tag="yout")
                        nc.vector.tensor_copy(out=yout[:, :], in_=ps2[:, :])
                        nc.scalar.dma_start(
                            out=yt_d[:, si, dc, lo:lo + ln], in_=yout[:, :],
                        )

            # Phased schedule: a long L1-only opening tile (first 3 subs),
            # then its L2 sweep, then the remaining subs pipelined. Keeping
            # the opening to a single engine class holds early chip power
            # down (the 50%-duty power-brake windows fire on concurrency
            # spikes) and gives the w2 transfer a wide landing window.
            t0 = min(3, nsub)
            for si in range(t0):
                layer1(si)
            for si in range(t0):
                layer2(si)
            for si in range(t0, nsub):
                layer1(si)
                layer2(si, col_split=2 if si == nsub - 1 else 1)

    nc.finalize()
    return nc


def kernel(hidden, top_k_indices, top_k_weights, W1, b1, W2, b2):
    global LAST_RESULTS
    x = np.ascontiguousarray(np.asarray(hidden, dtype=np.float32).reshape(T, D))
    idx = np.asarray(top_k_indices).reshape(T, TOPK)
    w = np.asarray(top_k_weights, dtype=np.float32).reshape(T, TOPK)
    W1 = np.asarray(W1, dtype=np.float32)
    b1 = np.asarray(b1, dtype=np.float32)
    W2 = np.asarray(W2, dtype=np.float32)
    b2 = np.asarray(b2, dtype=np.float32)

    # Host routing: token lists + combined weights per expert
    tok_lists, cw_lists = [], []
    for e in range(E):
        m = idx == e
        toks = np.nonzero(m.any(axis=1))[0]
        cw_t = (w * m).sum(axis=1)[toks]
        tok_lists.append(toks)
        cw_lists.append(cw_t)

    maxn = max(len(t) for t in tok_lists)
    C0 = max(512, -(-maxn // 64) * 64)
    nsub, csub, C = _grid(C0)

    if C not in _nc_cache:
        _nc_cache[C] = _build_nc(C)
    nc = _nc_cache[C]

    in_maps = []
    for e in range(E):
        toks = tok_lists[e]
        n = len(toks)
        xe = np.zeros((D, C), NPBF16)
        xe[:, :n] = x[toks].T.astype(NPBF16)
        in_maps.append({
            # [128, nsub, DC, csub]: xt[p, s, dc, t] = xe[dc*128+p, s*csub+t]
            "xt": np.ascontiguousarray(
                xe.reshape(DC, 128, nsub, csub).transpose(1, 2, 0, 3)
            ),
            # [128, FC, DC, 128]: w1[p, fc, dc, j] = W1e[dc*128+p, fc*128+j]
            "w1": np.ascontiguousarray(
                W1[e].astype(NPBF16).reshape(DC, 128, FC, 128).transpose(1, 2, 0, 3)
            ),
            # [128, DC, FC, 128]: w2[p, dc, fc, j] = W2e[fc*128+p, dc*128+j]
            "w2": np.ascontiguousarray(
                W2[e].astype(NPBF16).reshape(FC, 128, DC, 128).transpose(1, 2, 0, 3)
            ),
            # [128, FC]: b1[p, fc] = b1e[fc*128+p]
            "b1": np.ascontiguousarray(b1[e].reshape(FC, 128).T),
        })

    kwargs = {}
    if TRACE:
        kwargs = dict(trace=True, trace_cores=list(range(E)))
    res = run_bass_kernel_spmd(nc, in_maps, core_ids=list(range(E)), **kwargs)
    LAST_RESULTS = res

    out = np.zeros((T, D), np.float32)
    for e in range(E):
        toks = tok_lists[e]
        n = len(toks)
        yt = res.results[e]["yt"]  # [128, nsub, DC, csub] bf16
        y = yt.transpose(2, 0, 1, 3).reshape(D, C)[:, :n].astype(np.float32).T
        out[toks] += cw_lists[e][:, None] * y
        if b2[e].any():
            out[toks] += cw_lists[e][:, None] * b2[e][None, :]
    return out.reshape(B, S, D)



# revision 3
# speedup vs baseline: 1.0144x; 1.0144x over previous
"""Expert-parallel MoE layer for 8 Trainium2 NeuronCores.

Strategy: each of the 8 experts is assigned to one core. The host computes
the routing (which tokens go to which expert and with what combined weight),
gathers + transposes each expert's tokens into bf16 device tensors that are
pre-packed in the exact SBUF consumption layout (partition-major), and each
core runs a fused  gelu(x @ W1 + b1) @ W2  kernel for its expert. The host
applies the per-token combine weight and the (zero-ish) b2 term while
scatter-adding the per-expert outputs back into the full [B, S, D] output,
so neither cw nor b2 ever travels to the device.

Why pre-pack: a [D, C] activation layout gives the DMA engines 1KB runs and
512-row descriptors per tile; packed [128, DC, ssz] layouts give 128-row
descriptors with multi-KB contiguous runs per partition, which is what the
head of the kernel (PE waiting on the first tiles) is bound by. Tensor count
is kept low (5) because each bound DRAM tensor adds launch preamble time.

All matmul operands and the output travel as bf16 (norm rel err ~4e-3,
budget 2e-2): halves DMA bytes and SBUF pressure vs f32r at the same
1 column/cycle PE rate.

Schedule: token columns are processed in nsub equal sub-blocks (<=512 cols,
one PSUM bank) in a phased order -- L1 over the first three subs, then
their L2 sweep, then the remaining subs pipelined -- which keeps early chip
power to one engine class (the 50%-duty power-brake windows fire on
concurrency spikes) and gives w2 a wide landing window. The last sub's L2
runs in half-column passes so the kernel tail drains a short block. Inputs
feed through both HWDGE rings (sync: w1 in per-fc stages / x^T head / w2;
scalar: b1 + late x^T), outputs ride the scalar ring. Zero-operand PE-warmup
matmuls ramp the clock during the DMA wait while keeping pre-stream power
low (hot warmup on all 8 cores provokes a chip-wide downclock).
"""

import sys

if "/opt/trn_rl_repo" not in sys.path:
    sys.path.insert(0, "/opt/trn_rl_repo")

import ml_dtypes
import numpy as np

import concourse.bass as bass
import concourse.tile as tile
from concourse import bacc, mybir
from concourse.bass_utils import run_bass_kernel_spmd

B, S, D, F, E, TOPK = 4, 2048, 512, 1024, 8, 2
T = B * S
F32 = mybir.dt.float32
BF16 = mybir.dt.bfloat16
NPBF16 = ml_dtypes.bfloat16

DC = D // 128  # 4 contraction chunks for x @ W1
FC = F // 128  # 8 contraction chunks for h @ W2

# PE clock-ramp warmup: number of 128-col dummy matmuls issued before the
# first data-dependent matmul (tuned against the trace so the chain ends
# right as the first x^T/W1 slices land; slightly long is safer than idle).
N_WARMUP = 26

# Filler matmuls woven into the first L1 groups: they bridge early-DMA
# supply hiccups so the PE's HAM activity window never sees an idle gap
# (one >=0.3us gap resets the 3.4us busy window and holds the clock at
# 1.2GHz for another window). Keyed by (position); values = filler count.
FILL_IN_FC0 = 3   # between dc1 and dc2 of the first fc group (xt dc23 wait)
FILL_AFTER = [2, 1, 1, 0, 0, 0, 0, 0]  # after each fc group of sub 0

# Tail: the very last dc group of the last sub runs its final columns as a
# separate short chunk so the end-of-kernel cast+DMA chain drains quickly.
TAIL_COLS = 96

# Set by test harness to capture a profile; harness-invisible otherwise.
TRACE = False
LAST_RESULTS = None

_nc_cache = {}


def _grid(C):
    """(nsub, Csub, C_padded): equal sub-blocks of <=512 cols."""
    nsub = -(-C // 512)
    csub = -(-C // (nsub * 16)) * 16
    return nsub, csub, nsub * csub


def _build_nc(C):
    nsub, csub, cpad = _grid(C)
    assert cpad == C, (C, nsub, csub)

    nc = bacc.Bacc("TRN2", num_devices=E)

    xt_d = nc.dram_tensor("xt", [128, nsub, DC, csub], BF16, kind="ExternalInput")
    w1_d = nc.dram_tensor("w1", [128, FC, DC, 128], BF16, kind="ExternalInput")
    w2_d = nc.dram_tensor("w2", [128, DC, FC, 128], BF16, kind="ExternalInput")
    b1_d = nc.dram_tensor("b1", [128, FC], F32, kind="ExternalInput")
    yt_d = nc.dram_tensor("yt", [128, nsub, DC, csub], BF16, kind="ExternalOutput")

    with tile.TileContext(nc) as tc:
        with (
            tc.tile_pool(name="consts", bufs=1) as consts,
            tc.tile_pool(name="xtp", bufs=1) as xtp,
            tc.tile_pool(name="hp", bufs=26) as hp,
            tc.tile_pool(name="ybig", bufs=8) as ybigp,
            tc.tile_pool(name="ps_h", bufs=3, space="PSUM") as ps_h,
            tc.tile_pool(name="ps_y", bufs=4, space="PSUM") as ps_y,
            tc.tile_pool(name="ps_wu", bufs=1, space="PSUM") as ps_wu,
        ):
            # warmup operand memsets on two idle engines so both finish
            # right after the framework preamble and the warmup chain can
            # start ~0.5us earlier
            wu_w = consts.tile([128, 128], BF16, tag="wu_w")
            nc.gpsimd.memset(wu_w[:, :], 0.0)
            wu_x = consts.tile([128, 128], BF16, tag="wu_x")
            nc.vector.memset(wu_x[:, :], 0.0)
            wu_ps = ps_wu.tile([128, 128], F32, tag="pswu")
            for k in range(N_WARMUP):
                nc.tensor.matmul(
                    wu_ps[:, :], wu_w[:, :], wu_x[:, :],
                    start=(k == 0), stop=(k == N_WARMUP - 1),
                )

            def filler(n):
                # HAM-busy filler: single-MM groups into the dedicated
                # warmup PSUM bank (never read, no cross-engine deps)
                for _ in range(n):
                    nc.tensor.matmul(
                        wu_ps[:, :], wu_w[:, :], wu_x[:, :],
                        start=True, stop=True,
                    )

            # ---- head DMA: the critical first tiles split ACROSS both
            # HWDGE rings so each ring's ~1.6us cold-start runs in parallel
            # and the first fc group's operands land ~2us earlier.
            # sync ring:   w1 fc0 | xt s0 dc2-3 | w1 fc1..7 | xt s2 | w2
            # scalar ring: xt s0 dc0 | xt s0 dc1 | b1 | xt s1 | xt s3 | outs
            w1_sb = consts.tile([128, FC, DC, 128], BF16, tag="w1")
            nc.sync.dma_start(out=w1_sb[:, 0, :, :], in_=w1_d[:, 0, :, :])
            xt0b = xtp.tile([128, 2, csub], BF16, tag="xt0b", name="xt_sb0b")
            nc.sync.dma_start(out=xt0b[:, :, :], in_=xt_d[:, 0, 2:4, :])
            for fc in range(1, FC):
                nc.sync.dma_start(
                    out=w1_sb[:, fc, :, :], in_=w1_d[:, fc, :, :]
                )

            xt0a0 = xtp.tile([128, 1, csub], BF16, tag="xt0a0", name="xt_sb0a0")
            nc.scalar.dma_start(out=xt0a0[:, :, :], in_=xt_d[:, 0, 0:1, :])
            xt0a1 = xtp.tile([128, 1, csub], BF16, tag="xt0a1", name="xt_sb0a1")
            nc.scalar.dma_start(out=xt0a1[:, :, :], in_=xt_d[:, 0, 1:2, :])
            b1_sb = consts.tile([128, FC], F32, tag="b1")
            nc.scalar.dma_start(out=b1_sb[:, :], in_=b1_d[:, :])

            xt_sb = {}

            def _load_xt(si, eng):
                t = xtp.tile([128, DC, csub], BF16, tag=f"xt{si}", name=f"xt_sb{si}")
                eng.dma_start(out=t[:, :, :], in_=xt_d[:, si, :, :])
                xt_sb[si] = t

            if nsub > 1:
                _load_xt(1, nc.scalar)
            if nsub > 2:
                _load_xt(2, nc.sync)

            # w2 staged per-dc on sync behind w1/xt: each 0.5MB stage lands
            # just ahead of its L2 dc sweep
            w2_sb = consts.tile([128, DC, FC, 128], BF16, tag="w2")
            for dc in range(DC):
                nc.sync.dma_start(out=w2_sb[:, dc, :, :], in_=w2_d[:, dc, :, :])

            for si in range(3, nsub):
                _load_xt(si, nc.scalar)

            h_tiles = {}

            def layer1(si, with_fillers=False):
                for fc in range(FC):
                    ps = ps_h.tile([128, csub], F32, tag="psh")
                    for dc in range(DC):
                        if si == 0:
                            rhs = (xt0a0[:, 0, :] if dc == 0
                                   else xt0a1[:, 0, :] if dc == 1
                                   else xt0b[:, dc - 2, :])
                        else:
                            rhs = xt_sb[si][:, dc, :]
                        nc.tensor.matmul(
                            ps[:, :],
                            w1_sb[:, fc, dc, :],
                            rhs,
                            start=(dc == 0),
                            stop=(dc == DC - 1),
                        )
                        if with_fillers and fc == 0 and dc == 1:
                            filler(FILL_IN_FC0)
                    h = hp.tile([128, csub], BF16, tag="h")
                    nc.scalar.activation(
                        h[:, :], ps[:, :],
                        mybir.ActivationFunctionType.Gelu_apprx_tanh,
                        bias=b1_sb[:, fc:fc + 1],
                    )
                    h_tiles[(si, fc)] = h
                    if with_fillers:
                        filler(FILL_AFTER[fc])

            def layer2(si, tail=False):
                for dc in range(DC):
                    if tail and dc == DC - 1 and csub > 2 * TAIL_COLS:
                        chunks = [(0, csub - TAIL_COLS),
                                  (csub - TAIL_COLS, TAIL_COLS)]
                    else:
                        chunks = [(0, csub)]
                    for lo, ln in chunks:
                        ps2 = ps_y.tile([128, ln], F32, tag="psy")
                        for fc in range(FC):
                            nc.tensor.matmul(
                                ps2[:, :],
                                w2_sb[:, dc, fc, :],
                                h_tiles[(si, fc)][:, lo:lo + ln],
                                start=(fc == 0),
                                stop=(fc == FC - 1),
                            )
                        # PSUM -> bf16 SBUF on the DVE; combine weights and
                        # b2 are applied on the host during the scatter.
                        yout = ybigp.tile([128, ln], BF16, tag="yout")
                        nc.vector.tensor_copy(out=yout[:, :], in_=ps2[:, :])
                        nc.scalar.dma_start(
                            out=yt_d[:, si, dc, lo:lo + ln], in_=yout[:, :],
                        )

            # Phased schedule: a long L1-only opening tile (first 3 subs),
            # then its L2 sweep, then the remaining subs pipelined. Keeping
            # the opening to a single engine class holds early chip power
            # down (the 50%-duty power-brake windows fire on concurrency
            # spikes) and gives the w2 transfer a wide landing window.
            t0 = min(3, nsub)
            for si in range(t0):
                layer1(si, with_fillers=(si == 0))
            for si in range(t0):
                layer2(si)
            for si in range(t0, nsub):
                layer1(si)
                layer2(si, tail=si == nsub - 1)

    nc.finalize()
    return nc


def kernel(hidden, top_k_indices, top_k_weights, W1, b1, W2, b2):
    global LAST_RESULTS
    x = np.ascontiguousarray(np.asarray(hidden, dtype=np.float32).reshape(T, D))
    idx = np.asarray(top_k_indices).reshape(T, TOPK)
    w = np.asarray(top_k_weights, dtype=np.float32).reshape(T, TOPK)
    W1 = np.asarray(W1, dtype=np.float32)
    b1 = np.asarray(b1, dtype=np.float32)
    W2 = np.asarray(W2, dtype=np.float32)
    b2 = np.asarray(b2, dtype=np.float32)

    # Host routing: token lists + combined weights per expert
    tok_lists, cw_lists = [], []
    for e in range(E):
        m = idx == e
        toks = np.nonzero(m.any(axis=1))[0]
        cw_t = (w * m).sum(axis=1)[toks]
        tok_lists.append(toks)
        cw_lists.append(cw_t)

    maxn = max(len(t) for t in tok_lists)
    C0 = max(512, -(-maxn // 64) * 64)
    nsub, csub, C = _grid(C0)

    if C not in _nc_cache:
        _nc_cache[C] = _build_nc(C)
    nc = _nc_cache[C]

    in_maps = []
    for e in range(E):
        toks = tok_lists[e]
        n = len(toks)
        xe = np.zeros((D, C), NPBF16)
        xe[:, :n] = x[toks].T.astype(NPBF16)
        in_maps.append({
            # [128, nsub, DC, csub]: xt[p, s, dc, t] = xe[dc*128+p, s*csub+t]
            "xt": np.ascontiguousarray(
                xe.reshape(DC, 128, nsub, csub).transpose(1, 2, 0, 3)
            ),
            # [128, FC, DC, 128]: w1[p, fc, dc, j] = W1e[dc*128+p, fc*128+j]
            "w1": np.ascontiguousarray(
                W1[e].astype(NPBF16).reshape(DC, 128, FC, 128).transpose(1, 2, 0, 3)
            ),
            # [128, DC, FC, 128]: w2[p, dc, fc, j] = W2e[fc*128+p, dc*128+j]
            "w2": np.ascontiguousarray(
                W2[e].astype(NPBF16).reshape(FC, 128, DC, 128).transpose(1, 2, 0, 3)
            ),
            # [128, FC]: b1[p, fc] = b1e[fc*128+p]
            "b1": np.ascontiguousarray(b1[e].reshape(FC, 128).T),
        })

    kwargs = {}
    if TRACE:
        kwargs = dict(trace=True, trace_cores=list(range(E)))
    res = run_bass_kernel_spmd(nc, in_maps, core_ids=list(range(E)), **kwargs)
    LAST_RESULTS = res

    out = np.zeros((T, D), np.float32)
    for e in range(E):
        toks = tok_lists[e]
        n = len(toks)
        yt = res.results[e]["yt"]  # [128, nsub, DC, csub] bf16
        y = yt.transpose(2, 0, 1, 3).reshape(D, C)[:, :n].astype(np.float32).T
        out[toks] += cw_lists[e][:, None] * y
        if b2[e].any():
            out[toks] += cw_lists[e][:, None] * b2[e][None, :]
    return out.reshape(B, S, D)

